# revision 6
# baseline (speedup 1.0000x reference)
"""Trainium2 Bass kernel for nn_CAModule (channel attention, sparse_attention).

Reference computation per batch b (x: [16, 512, 64, 64] f32, beta: [1] f32):
    q = x[b].reshape(512, 4096)              # [C, N]
    energy = q @ q.T                         # [C, C]   (symmetric!)
    att = softmax(max_j(energy) - energy)    # row-wise, == softmax(-energy)
    out[b] = beta * (att @ q)                # [C, N]

Sharding: data-parallel over batch, 2 batches per core on 8 cores.

Key tricks:
  - softmax(max - energy) == exp(mn_i - e_ij)/Z_i with mn_i = row min
    (shift invariance; mn is the max of the softmax argument).
  - energy is symmetric, so att^T (needed as the stationary operand of the
    second matmul) is computed directly from the energy tiles: the tile of
    rows jc is also the tile of columns jc. Only q itself needs a physical
    512x4096 transpose (done on the PE via identity matmuls).
  - mn_i is subtracted along the *free* dim of the transposed tiles by a
    K=1 accumulating matmul ((-1s) x mnT) into the energy PSUM banks.
  - matmuls run as float32r (e8m11, RNE-on-write, exact PE): 1 cycle/row
    vs 4 for f32. Mode "split" decomposes q = h + l (h = f32r(q)) and runs
    E = h@hT + h@lT + l@hT for ~fp32 accuracy at 3 passes.

Modes via CAM_MODE env: "f32r" (default), "split", "f32".
"""
import os
import sys

sys.path.insert(0, "/opt/trn_rl_repo")

import numpy as np  # noqa: E402

try:
    import jax

    jax.config.update("jax_compilation_cache_dir", "/tmp/jax_cc_cache")
    jax.config.update("jax_persistent_cache_min_compile_time_secs", 0.0)
except Exception:
    pass

import concourse.bass as bass  # noqa: E402
import concourse.bacc as bacc  # noqa: E402
import concourse.mybir as mybir  # noqa: E402
from concourse.tile import TileContext  # noqa: E402
from concourse.masks import make_identity  # noqa: E402
from concourse.bass_utils import run_bass_kernel_spmd  # noqa: E402

F32 = mybir.dt.float32
F32R = mybir.dt.float32r
AX = mybir.AxisListType
OP = mybir.AluOpType
AF = mybir.ActivationFunctionType

B, C, HH, WW = 16, 512, 64, 64
N = HH * WW          # 4096
P = 128
NCORES = 8
BPC = B // NCORES    # 2 batches per core
CC = C // P          # 4 channel chunks
NT = N // P          # 32 spatial chunks (transpose granularity)
NF = N // 512        # 8 output free-dim chunks

MODE = os.environ.get("CAM_MODE", "f32r")


def build_nc(mode: str, bpc: int = BPC):
    nc = bacc.Bacc(None, target_bir_lowering=False)
    xs = nc.dram_tensor("xs", [bpc, C, N], F32, kind="ExternalInput")
    beta = nc.dram_tensor("beta", [1, 1], F32, kind="ExternalInput")
    ys = nc.dram_tensor("ys", [bpc, C, N], F32, kind="ExternalOutput")

    # matmul dtype for the two big matmuls
    MMDT = F32 if mode == "f32" else F32R
    # dtype in which q is loaded / transposed
    QDT = F32R if mode == "f32r" else F32
    # dtype of the mn-fold matmul operands (exact f32 unless pure-f32r mode)
    NDT = F32R if mode == "f32r" else F32

    with TileContext(nc) as tc:
        with (
            tc.tile_pool(name="consts", bufs=1) as consts,
            tc.tile_pool(name="pq", bufs=(4 if mode == "split" else 8)) as pq,
            tc.tile_pool(name="pqr", bufs=4) as pqr,
            tc.tile_pool(name="pqt", bufs=3) as pqt,
            tc.tile_pool(name="pexpt", bufs=8) as pexpt,
            tc.tile_pool(name="pscr", bufs=2) as pscr,
            tc.tile_pool(name="posb", bufs=4) as posb,
            tc.tile_pool(name="pstat", bufs=2) as pstat,
            tc.tile_pool(name="pse", bufs=4, space="PSUM") as pse,
            tc.tile_pool(name="psw", bufs=3, space="PSUM") as psw,
            tc.tile_pool(name="pss", bufs=1, space="PSUM") as pss,
        ):
            # ---- constants ----
            ident = consts.tile([P, P], F32)
            make_identity(nc, ident)
            if QDT == F32R:
                identq = consts.tile([P, P], F32R)
                nc.vector.tensor_copy(identq, ident)
            else:
                identq = ident
            ones1 = consts.tile([1, P], F32)
            nc.vector.memset(ones1, 1.0)
            negones_f = consts.tile([1, P], F32)
            nc.vector.memset(negones_f, -1.0)
            if NDT == F32R:
                negones = consts.tile([1, P], F32R)
                nc.vector.tensor_copy(negones, negones_f)
            else:
                negones = negones_f

            # beta broadcast to [P, 1]
            beta_sb = consts.tile([1, 1], F32)
            nc.sync.dma_start(beta_sb, beta[:, :])
            ps_b = pss.tile([P, 1], F32, tag="s")
            nc.tensor.matmul(ps_b, ones1, beta_sb, start=True, stop=True)
            beta_bc = consts.tile([P, 1], F32)
            nc.vector.tensor_copy(beta_bc, ps_b)

            for b in range(bpc):
                # ---- load q ----
                Q = []
                for c in range(CC):
                    q = pq.tile([P, N], QDT, tag="q")
                    src = xs[b, P * c : P * (c + 1), :]
                    if QDT == F32R:
                        nc.gpsimd.dma_start(q, src)   # SWDGE cast f32->f32r
                    else:
                        nc.sync.dma_start(q, src)
                    Q.append(q)
                if mode == "split":
                    Qr = []
                    for c in range(CC):
                        qr = pqr.tile([P, N], F32R, tag="qr")
                        nc.gpsimd.dma_start(qr, xs[b, P * c : P * (c + 1), :])
                        Qr.append(qr)
                else:
                    Qr = Q

                # ---- energy: E[ic] = (q @ q.T)[ic-chunk, :] via transposed tiles ----
                E = [
                    pse.tile([P, 512], F32, tag="e", name=f"E{b}_{i}")
                    for i in range(CC)
                ]
                for t in range(NT):
                    stg = psw.tile([P, 512], QDT, tag="w")
                    for c in range(CC):
                        nc.tensor.transpose(
                            stg[:, P * c : P * (c + 1)],
                            Q[c][:, P * t : P * (t + 1)],
                            identq,
                        )
                    if mode == "split":
                        ht = pqt.tile([P, 512], F32R, tag="ht")
                        lt = pqt.tile([P, 512], F32R, tag="lt")
                        nc.vector.tensor_copy(ht, stg)          # h = rne11(q)
                        nc.vector.tensor_tensor(
                            lt, stg, ht.bitcast(F32), op=OP.subtract
                        )                                        # l = q - h
                        ops = [(ht, ht), (ht, lt), (lt, ht)]
                    else:
                        qt = pqt.tile([P, 512], MMDT, tag="qt")
                        nc.vector.tensor_copy(qt, stg)
                        ops = [(qt, qt)]
                    for oi, (L, R) in enumerate(ops):
                        for ic in range(CC):
                            nc.tensor.matmul(
                                E[ic],
                                L[:, P * ic : P * (ic + 1)],
                                R,
                                start=(t == 0 and oi == 0),
                                stop=(t == NT - 1 and oi == len(ops) - 1),
                            )

                # ---- row stats: mn = rowmin(E), Z = sum_j exp(mn - e) ----
                mn = pstat.tile([P, CC], F32, tag="mn")
                for ic in range(CC):
                    nc.vector.tensor_reduce(
                        mn[:, ic : ic + 1], E[ic], axis=AX.X, op=OP.min
                    )
                if mode == "f32r":
                    mnv = pstat.tile([P, CC], F32R, tag="mnv")
                    nc.vector.tensor_copy(mnv, mn)  # rne11 so matmul sees same value
                    mn_bias = mnv.bitcast(F32)
                    tsrc = mnv
                else:
                    mn_bias = mn
                    tsrc = mn

                Z = pstat.tile([P, CC], F32, tag="z")
                for ic in range(CC):
                    scr = pscr.tile([P, 512], F32, tag="scr")
                    nc.scalar.activation(
                        scr,
                        E[ic],
                        AF.Exp,
                        bias=mn_bias[:, ic : ic + 1],
                        scale=-1.0,
                        accum_out=Z[:, ic : ic + 1],
                    )

                # ---- mnT: [1, 512] row vector of mn ----
                ps_s = pss.tile([CC, P], NDT, tag="s")
                nc.tensor.matmul(ps_s, tsrc, identq, is_transpose=True, start=True, stop=True)
                sbs = pstat.tile([CC, P], NDT, tag="sbs")
                nc.vector.tensor_copy(sbs, ps_s)
                mnT = pstat.tile([1, C], NDT, tag="mnT")
                for c in range(CC):
                    nc.sync.dma_start(
                        mnT[0:1, P * c : P * (c + 1)], sbs[c : c + 1, :]
                    )

                # ---- fold -mn along free dim into E (E' = e[j,i] - mn_i) ----
                for ic in range(CC):
                    nc.tensor.matmul(
                        E[ic], negones, mnT,
                        start=False, stop=True, skip_group_check=True,
                    )

                # ---- att^T tiles: expT[jc][j, i] = exp(mn_i - e[j, i]) ----
                expT = []
                for jc in range(CC):
                    eT = pexpt.tile([P, C], MMDT, tag="expt")
                    nc.scalar.activation(eT, E[jc], AF.Exp, scale=-1.0)
                    expT.append(eT)

                # ---- scale vector: rZb = beta / Z ----
                rZ = pstat.tile([P, CC], F32, tag="rz")
                nc.vector.reciprocal(rZ, Z)
                rZb = pstat.tile([P, CC], F32, tag="rzb")
                nc.vector.tensor_tensor(
                    rZb, rZ, beta_bc.broadcast_to([P, CC]), op=OP.mult
                )

                # ---- out = rZb * (expT.T @ q) ----
                for ic in range(CC):
                    for nf in range(NF):
                        po = psw.tile([P, 512], F32, tag="w")
                        for jc in range(CC):
                            nc.tensor.matmul(
                                po,
                                expT[jc][:, P * ic : P * (ic + 1)],
                                Qr[jc][:, 512 * nf : 512 * (nf + 1)],
                                start=(jc == 0),
                                stop=(jc == CC - 1),
                            )
                        ob = posb.tile([P, 512], F32, tag="osb")
                        nc.scalar.activation(
                            ob, po, AF.Copy, scale=rZb[:, ic : ic + 1]
                        )
                        nc.sync.dma_start(
                            ys[b, P * ic : P * (ic + 1), 512 * nf : 512 * (nf + 1)],
                            ob,
                        )
    nc.finalize()
    return nc


_NC_CACHE = {}


def _get_nc(mode: str, bpc: int = BPC):
    key = (mode, bpc)
    if key not in _NC_CACHE:
        _NC_CACHE[key] = build_nc(mode, bpc)
    return _NC_CACHE[key]


def kernel(x: np.ndarray, beta: np.ndarray) -> np.ndarray:
    x = np.ascontiguousarray(np.asarray(x, dtype=np.float32))
    beta2 = np.asarray(beta, dtype=np.float32).reshape(1, 1)
    assert x.shape == (B, C, HH, WW)
    xf = x.reshape(B, C, N)

    nc = _get_nc(MODE)
    in_maps = [
        {"xs": xf[k * BPC : (k + 1) * BPC], "beta": beta2} for k in range(NCORES)
    ]
    res = run_bass_kernel_spmd(nc, in_maps, list(range(NCORES)))
    out = np.concatenate([r["ys"] for r in res.results], axis=0)
    return out.reshape(B, C, HH, WW).astype(np.float32, copy=False)


if __name__ == "__main__":
    rng = np.random.default_rng(0)
    x = rng.standard_normal((B, C, HH, WW), dtype=np.float32)
    beta = rng.standard_normal(1).astype(np.float32)
    y = kernel(x=x, beta=beta)
    print("out", y.shape, y.dtype, float(np.abs(y).max()))


# revision 10
# speedup vs baseline: 4453.6588x; 4453.6588x over previous
"""Trainium2 Bass kernel for nn_CAModule (channel attention, sparse_attention).

Reference computation per batch b (x: [16, 512, 64, 64] f32, beta: [1] f32):
    q = x[b].reshape(512, 4096)              # [C, N]
    energy = q @ q.T                         # [C, C]   (symmetric!)
    att = softmax(max_j(energy) - energy)    # row-wise, == softmax(-energy)
    out[b] = beta * (att @ q)                # [C, N]

Sharding: data-parallel over batch, 2 batches per core on 8 cores.

Key tricks:
  - softmax(max - energy) == exp(mn_i - e_ij)/Z_i with mn_i = row min
    (shift invariance; mn is the max of the softmax argument).
  - energy is symmetric, so att^T (needed as the stationary operand of the
    second matmul) is computed directly from the energy tiles: the tile of
    rows jc is also the tile of columns jc. Only q itself needs a physical
    512x4096 transpose (done on the PE via identity matmuls).
  - mn_i is subtracted along the *free* dim of the transposed tiles by a
    K=1 accumulating matmul ((-1s) x mnT) into the energy PSUM banks.
  - matmuls run as float32r (e8m11, RNE-on-write, exact PE): 1 cycle/row
    vs 4 for f32. Mode "split" decomposes q = h + l (h = f32r(q)) and runs
    E = h@hT + h@lT + l@hT for ~fp32 accuracy at 3 passes.

Modes via CAM_MODE env: "f32r" (default), "split", "f32".
"""
import os
import sys

sys.path.insert(0, "/opt/trn_rl_repo")

import numpy as np  # noqa: E402

try:
    import jax

    jax.config.update("jax_compilation_cache_dir", "/tmp/jax_cc_cache")
    jax.config.update("jax_persistent_cache_min_compile_time_secs", 0.0)
except Exception:
    pass

import concourse.bass as bass  # noqa: E402
import concourse.bacc as bacc  # noqa: E402
import concourse.mybir as mybir  # noqa: E402
from concourse.tile import TileContext  # noqa: E402
from concourse.masks import make_identity  # noqa: E402
from concourse.bass_utils import run_bass_kernel_spmd  # noqa: E402

F32 = mybir.dt.float32
F32R = mybir.dt.float32r
AX = mybir.AxisListType
OP = mybir.AluOpType
AF = mybir.ActivationFunctionType

B, C, HH, WW = 16, 512, 64, 64
N = HH * WW          # 4096
P = 128
NCORES = 8
BPC = B // NCORES    # 2 batches per core
CC = C // P          # 4 channel chunks
NT = N // P          # 32 spatial chunks (transpose granularity)
NF = N // 512        # 8 output free-dim chunks

MODE = os.environ.get("CAM_MODE", "f32r")


def build_nc(mode: str, bpc: int = BPC, reps: int = 1):
    nc = bacc.Bacc(None, target_bir_lowering=False)
    xs = nc.dram_tensor("xs", [bpc, C, N], F32, kind="ExternalInput")
    beta = nc.dram_tensor("beta", [1, 1], F32, kind="ExternalInput")
    ys = nc.dram_tensor("ys", [bpc, C, N], F32, kind="ExternalOutput")

    # matmul dtype for the two big matmuls
    MMDT = F32 if mode == "f32" else F32R
    # dtype in which q is loaded / transposed
    QDT = F32R if mode == "f32r" else F32
    # dtype of the mn-fold matmul operands (exact f32 unless pure-f32r mode)
    NDT = F32R if mode == "f32r" else F32

    with TileContext(nc) as tc:
        with (
            tc.tile_pool(name="consts", bufs=1) as consts,
            tc.tile_pool(name="pq", bufs=(4 if mode == "split" else 8)) as pq,
            tc.tile_pool(name="pqr", bufs=4) as pqr,
            tc.tile_pool(name="pqt", bufs=3) as pqt,
            tc.tile_pool(name="pexpt", bufs=8) as pexpt,
            tc.tile_pool(name="pscr", bufs=2) as pscr,
            tc.tile_pool(name="posb", bufs=4) as posb,
            tc.tile_pool(name="pstat", bufs=2) as pstat,
            tc.tile_pool(name="pse", bufs=4, space="PSUM") as pse,
            tc.tile_pool(name="psw", bufs=3, space="PSUM") as psw,
            tc.tile_pool(name="pss", bufs=1, space="PSUM") as pss,
        ):
            # ---- constants ----
            ident = consts.tile([P, P], F32)
            make_identity(nc, ident)
            if QDT == F32R:
                identq = consts.tile([P, P], F32R)
                nc.vector.tensor_copy(identq, ident)
            else:
                identq = ident
            ones1 = consts.tile([1, P], F32)
            nc.vector.memset(ones1, 1.0)
            negones_f = consts.tile([1, P], F32)
            nc.vector.memset(negones_f, -1.0)
            if NDT == F32R:
                negones = consts.tile([1, P], F32R)
                nc.vector.tensor_copy(negones, negones_f)
            else:
                negones = negones_f

            # beta broadcast to [P, 1]
            beta_sb = consts.tile([1, 1], F32)
            nc.sync.dma_start(beta_sb, beta[:, :])
            ps_b = pss.tile([P, 1], F32, tag="s")
            nc.tensor.matmul(ps_b, ones1, beta_sb, start=True, stop=True)
            beta_bc = consts.tile([P, 1], F32)
            nc.vector.tensor_copy(beta_bc, ps_b)

            for b_rep in range(bpc * reps):
                b = b_rep % bpc
                # ---- load q ----
                Q = []
                for c in range(CC):
                    q = pq.tile([P, N], QDT, tag="q")
                    src = xs[b, P * c : P * (c + 1), :]
                    if QDT == F32R:
                        nc.gpsimd.dma_start(q, src)   # SWDGE cast f32->f32r
                    else:
                        nc.sync.dma_start(q, src)
                    Q.append(q)
                if mode == "split":
                    Qr = []
                    for c in range(CC):
                        qr = pqr.tile([P, N], F32R, tag="qr")
                        nc.gpsimd.dma_start(qr, xs[b, P * c : P * (c + 1), :])
                        Qr.append(qr)
                else:
                    Qr = Q

                # ---- energy: E[ic] = (q @ q.T)[ic-chunk, :] via transposed tiles ----
                E = [
                    pse.tile([P, 512], F32, tag="e", name=f"E{b_rep}_{i}")
                    for i in range(CC)
                ]
                for t in range(NT):
                    stg = psw.tile([P, 512], QDT, tag="w")
                    for c in range(CC):
                        nc.tensor.transpose(
                            stg[:, P * c : P * (c + 1)],
                            Q[c][:, P * t : P * (t + 1)],
                            identq,
                        )
                    if mode == "split":
                        ht = pqt.tile([P, 512], F32R, tag="ht")
                        lt = pqt.tile([P, 512], F32R, tag="lt")
                        nc.vector.tensor_copy(ht, stg)          # h = rne11(q)
                        nc.vector.tensor_tensor(
                            lt, stg, ht.bitcast(F32), op=OP.subtract
                        )                                        # l = q - h
                        ops = [(ht, ht), (ht, lt), (lt, ht)]
                    else:
                        qt = pqt.tile([P, 512], MMDT, tag="qt")
                        nc.vector.tensor_copy(qt, stg)
                        ops = [(qt, qt)]
                    for oi, (L, R) in enumerate(ops):
                        for ic in range(CC):
                            nc.tensor.matmul(
                                E[ic],
                                L[:, P * ic : P * (ic + 1)],
                                R,
                                start=(t == 0 and oi == 0),
                                stop=(t == NT - 1 and oi == len(ops) - 1),
                            )

                # ---- row stats: mn = rowmin(E), Z = sum_j exp(mn - e) ----
                mn = pstat.tile([P, CC], F32, tag="mn")
                for ic in range(CC):
                    nc.vector.tensor_reduce(
                        mn[:, ic : ic + 1], E[ic], axis=AX.X, op=OP.min
                    )
                if mode == "f32r":
                    mnv = pstat.tile([P, CC], F32R, tag="mnv")
                    nc.vector.tensor_copy(mnv, mn)  # rne11 so matmul sees same value
                    mn_bias = mnv.bitcast(F32)
                    tsrc = mnv
                else:
                    mn_bias = mn
                    tsrc = mn

                Z = pstat.tile([P, CC], F32, tag="z")
                for ic in range(CC):
                    scr = pscr.tile([P, 512], F32, tag="scr")
                    nc.scalar.activation(
                        scr,
                        E[ic],
                        AF.Exp,
                        bias=mn_bias[:, ic : ic + 1],
                        scale=-1.0,
                        accum_out=Z[:, ic : ic + 1],
                    )

                # ---- mnT: [1, 512] row vector of mn ----
                ps_s = pss.tile([CC, P], NDT, tag="s")
                nc.tensor.matmul(ps_s, tsrc, identq, is_transpose=True, start=True, stop=True)
                sbs = pstat.tile([CC, P], NDT, tag="sbs")
                nc.vector.tensor_copy(sbs, ps_s)
                mnT = pstat.tile([1, C], NDT, tag="mnT")
                for c in range(CC):
                    nc.sync.dma_start(
                        mnT[0:1, P * c : P * (c + 1)], sbs[c : c + 1, :]
                    )

                # ---- fold -mn along free dim into E (E' = e[j,i] - mn_i) ----
                for ic in range(CC):
                    nc.tensor.matmul(
                        E[ic], negones, mnT,
                        start=False, stop=True, skip_group_check=True,
                    )

                # ---- att^T tiles: expT[jc][j, i] = exp(mn_i - e[j, i]) ----
                expT = []
                for jc in range(CC):
                    eT = pexpt.tile([P, C], MMDT, tag="expt")
                    nc.scalar.activation(eT, E[jc], AF.Exp, scale=-1.0)
                    expT.append(eT)

                # ---- scale vector: rZb = beta / Z ----
                rZ = pstat.tile([P, CC], F32, tag="rz")
                nc.vector.reciprocal(rZ, Z)
                rZb = pstat.tile([P, CC], F32, tag="rzb")
                nc.vector.tensor_tensor(
                    rZb, rZ, beta_bc.broadcast_to([P, CC]), op=OP.mult
                )

                # ---- out = rZb * (expT.T @ q) ----
                for ic in range(CC):
                    for nf in range(NF):
                        po = psw.tile([P, 512], F32, tag="w")
                        for jc in range(CC):
                            nc.tensor.matmul(
                                po,
                                expT[jc][:, P * ic : P * (ic + 1)],
                                Qr[jc][:, 512 * nf : 512 * (nf + 1)],
                                start=(jc == 0),
                                stop=(jc == CC - 1),
                            )
                        ob = posb.tile([P, 512], F32, tag="osb")
                        nc.scalar.activation(
                            ob, po, AF.Copy, scale=rZb[:, ic : ic + 1]
                        )
                        nc.sync.dma_start(
                            ys[b, P * ic : P * (ic + 1), 512 * nf : 512 * (nf + 1)],
                            ob,
                        )
    nc.finalize()
    return nc


_NC_CACHE = {}


def _get_nc(mode: str, bpc: int = BPC, reps: int = 1):
    key = (mode, bpc, reps)
    if key not in _NC_CACHE:
        _NC_CACHE[key] = build_nc(mode, bpc, reps)
    return _NC_CACHE[key]


def kernel(x: np.ndarray, beta: np.ndarray) -> np.ndarray:
    x = np.ascontiguousarray(np.asarray(x, dtype=np.float32))
    beta2 = np.asarray(beta, dtype=np.float32).reshape(1, 1)
    assert x.shape == (B, C, HH, WW)
    xf = x.reshape(B, C, N)

    nc = _get_nc(MODE)
    in_maps = [
        {"xs": xf[k * BPC : (k + 1) * BPC], "beta": beta2} for k in range(NCORES)
    ]
    res = run_bass_kernel_spmd(nc, in_maps, list(range(NCORES)))
    out = np.concatenate([r["ys"] for r in res.results], axis=0)
    return out.reshape(B, C, HH, WW).astype(np.float32, copy=False)


if __name__ == "__main__":
    rng = np.random.default_rng(0)
    x = rng.standard_normal((B, C, HH, WW), dtype=np.float32)
    beta = rng.standard_normal(1).astype(np.float32)
    y = kernel(x=x, beta=beta)
    print("out", y.shape, y.dtype, float(np.abs(y).max()))
